# revision 1
# baseline (speedup 1.0000x reference)
"""DiffusionBlock TRN2 kernel: spectral diffusion + sparse COO gradient op +
MLP + residual LayerNorm, sharded over 8 NeuronCores by node rows.

Self-contained: hardcodes all shapes; builds + compiles a Bass program at
call time (specialized to the edge distribution), runs SPMD on cores 0-7.
"""
import sys
sys.path.insert(0, '/opt/trn_rl_repo')
import numpy as np
import concourse.mybir as mybir
from concourse.bass import Bass
from concourse.tile import TileContext
from concourse import bass_utils, library_config

dt = mybir.dt

# problem dims (hardcoded per contract)
N, C, K, G, E = 65536, 256, 128, 32, 2097152
LN_EPS = 1e-5
NCORES = 8
RPC = N // NCORES          # rows per core = 8192
GPC = G // NCORES          # graphs per core = 4
NPG = N // G               # nodes per graph = 2048
NBLK = RPC // 128          # 128-row blocks per core = 64
BPG = NPG // 128           # blocks per graph = 16
HALF = 32768               # int16 gather index limit


# ---------------------------------------------------------------- BIR fixups
_wspill = [0]


def _legalize_waits(nc):
    """This walrus accepts at most 1 sync-wait per instruction (2 for
    EventSemaphore). Spill extras into EventSemaphore insts inserted just
    before, same engine. Also run codegen_inst_isa_subclasses (Bacc does it,
    raw Bass doesn't) so extended-ISA insts get their raw words."""
    mybir.codegen_inst_isa_subclasses(nc)
    f = nc.m.functions[0]
    for bb in f.blocks:
        out = []
        changed = False
        for ins in bb.instructions:
            si = ins.sync_info
            cap = 2 if ins.opcode == 'EventSemaphore' else 1
            if si is not None and si.on_wait is not None and len(si.on_wait) > cap:
                waits = list(si.on_wait)
                keep, spill = waits[:cap], waits[cap:]
                while spill:
                    batch, spill = spill[:2], spill[2:]
                    _wspill[0] += 1
                    es = mybir.InstEventSemaphore(
                        name=f"WSPILL-{_wspill[0]}", ins=[], outs=[])
                    es.engine = ins.engine
                    es.sync_info = mybir.SyncInfo(on_wait=batch, on_update=[])
                    out.append(es)
                si.on_wait = keep
                changed = True
            out.append(ins)
        if changed:
            bb.instructions = out
    return nc


def _calls_of(nch):
    """Split nch chunks into gather calls of at most 8 chunks each."""
    calls = []
    left = nch
    while left > 0:
        c = min(8, left)
        calls.append(c)
        left -= c
    return calls


# ---------------------------------------------------------------- host prep
def _prepare(inputs):
    x = np.asarray(inputs["x"], np.float32)
    evals = np.asarray(inputs["evals_batch"], np.float32)
    evecs = np.asarray(inputs["evecs"], np.float32)
    mass = np.asarray(inputs["mass"], np.float32)
    row = np.asarray(inputs["row"]).astype(np.int64)
    col = np.asarray(inputs["col"]).astype(np.int64)
    vals = np.asarray(inputs["vals"], np.float32)
    t_params = np.asarray(inputs["t_params"], np.float32)
    grad_W = np.asarray(inputs["grad_W"], np.float32)
    grad_b = np.asarray(inputs["grad_b"], np.float32)
    W1 = np.asarray(inputs["W1"], np.float32)
    b1 = np.asarray(inputs["b1"], np.float32)
    W2 = np.asarray(inputs["W2"], np.float32)
    b2 = np.asarray(inputs["b2"], np.float32)
    ln_g = np.asarray(inputs["ln_g"], np.float32)
    ln_b = np.asarray(inputs["ln_b"], np.float32)

    x16_full = x.astype(np.float16)

    # fold grad_W / grad_b into the second half of W1 (host, fp64 for accuracy)
    W1a = W1[:, :C]
    W1b = W1[:, C:]
    Wfold = (W1b.astype(np.float64) @ grad_W.astype(np.float64)).astype(np.float32)
    b1f_np = b1 + (W1b.astype(np.float64) @ grad_b.astype(np.float64)).astype(np.float32)

    # decay[g,k,c] = exp(-|t_c| * max(ev_gk, 0))
    t = np.abs(t_params)
    ev = np.maximum(evals.reshape(G, K), 0.0)
    decay = np.exp(-ev[:, :, None] * t[None, None, :]).astype(np.float32)  # [G,K,C]

    em_full = (evecs * mass[:, None]).astype(np.float16)   # [N,K]
    ev16_full = evecs.astype(np.float16)

    # ---- edge partitioning by destination row ----
    core_of = row >> 13               # row // 8192
    percore = []
    nlo = np.zeros((NCORES, NBLK), np.int64)
    nhi = np.zeros((NCORES, NBLK), np.int64)
    for i in range(NCORES):
        sel = np.where(core_of == i)[0]
        r = row[sel] - i * RPC
        c_ = col[sel]
        v = vals[sel]
        blk = r >> 7
        lo = c_ < HALF
        lists = []
        for b in range(NBLK):
            m = blk == b
            ilo = np.where(m & lo)[0]
            ihi = np.where(m & ~lo)[0]
            lists.append((ilo, ihi))
            nlo[i, b] = len(ilo)
            nhi[i, b] = len(ihi)
        percore.append((r, c_, v, lists))

    # per-block chunk counts, uniform across cores (SPMD)
    CLo_b = ((nlo.max(0) + 127) // 128).astype(np.int64)
    CHi_b = ((nhi.max(0) + 127) // 128).astype(np.int64)
    CT_b = CLo_b + CHi_b
    CTmax = int(CT_b.max())
    CTsum = int(CT_b.sum())
    blk_off = np.concatenate([[0], np.cumsum(CT_b)]).astype(np.int64)

    # call structure per block: list of (half, nchunks)
    call_plan = []
    for b in range(NBLK):
        plan = [(0, nc_) for nc_ in _calls_of(int(CLo_b[b]))] + \
               [(1, nc_) for nc_ in _calls_of(int(CHi_b[b]))]
        call_plan.append(plan)
    calls_per_blk = [len(p) for p in call_plan]
    call_off = np.concatenate([[0], np.cumsum(calls_per_blk)]).astype(np.int64)
    TOTCALL = int(call_off[-1])

    in_maps = []
    for i in range(NCORES):
        r, c_, v, lists = percore[i]
        gidx = np.zeros((16, TOTCALL, 64), np.int16)
        lrow16 = np.full((128, CTsum), 255.0, np.float16)
        vals32 = np.zeros((128, CTsum), np.float32)
        for b in range(NBLK):
            ncalls_lo = len(_calls_of(int(CLo_b[b])))
            for half, idxs_half, nch in ((0, lists[b][0], int(CLo_b[b])),
                                         (1, lists[b][1], int(CHi_b[b]))):
                cc = c_[idxs_half] - half * HALF
                rr = r[idxs_half] & 127
                vv = v[idxs_half]
                ne = len(cc)
                npad = nch * 128
                ccp = np.zeros(npad, np.int64)
                ccp[:ne] = cc
                rrp = np.full(npad, 255.0, np.float32)
                rrp[:ne] = rr
                vvp = np.zeros(npad, np.float32)
                vvp[:ne] = vv
                base_ct = int(blk_off[b]) + half * int(CLo_b[b])
                call0 = int(call_off[b]) + (0 if half == 0 else ncalls_lo)
                for ch in range(nch):
                    sl = slice(ch * 128, (ch + 1) * 128)
                    lrow16[:, base_ct + ch] = rrp[sl].astype(np.float16)
                    vals32[:, base_ct + ch] = vvp[sl]
                    call = call0 + ch // 8
                    j0 = (ch % 8) * 128
                    jj = np.arange(128) + j0
                    gidx[jj % 16, call, jj // 16] = ccp[sl].astype(np.int16)
        gidx = np.tile(gidx, (8, 1, 1))   # replicate for the 8 q7 cores

        sl_rows = slice(i * RPC, (i + 1) * RPC)
        sl_g = slice(i * GPC, (i + 1) * GPC)
        evT16 = np.ascontiguousarray(
            ev16_full[sl_rows].reshape(GPC, NPG, K).transpose(0, 2, 1)
        ).reshape(GPC * K, NPG)

        L16 = 1792 + CTmax * 128
        c16 = np.zeros((128, L16), np.float16)
        off = 0
        W1aT = W1a.T.astype(np.float16)
        WfT = Wfold.T.astype(np.float16)
        for Wt in (W1aT, WfT):
            for k in range(2):
                for m in range(2):
                    c16[:, off:off + 128] = Wt[k * 128:(k + 1) * 128,
                                               m * 128:(m + 1) * 128]
                    off += 128
        W2T = W2.T.astype(np.float16)
        for k in range(2):
            c16[:, off:off + 256] = W2T[k * 128:(k + 1) * 128, :]
            off += 256
        c16[:, off:off + 128] = np.eye(128, dtype=np.float16)
        off += 128
        c16[:, off:off + 128] = np.broadcast_to(
            np.arange(128, dtype=np.float16), (128, 128))
        off += 128
        c16[:, off:off + CTmax * 128] = np.broadcast_to(
            np.tile(np.arange(128, dtype=np.float16), CTmax), (128, CTmax * 128))
        off += CTmax * 128
        assert off == L16

        c32 = np.zeros((128, 516), np.float32)
        c32[:, 0] = b1f_np[:128]
        c32[:, 1] = b1f_np[128:]
        c32[:, 2:258] = np.broadcast_to(ln_g, (128, C))
        c32[:, 258:514] = np.broadcast_to(ln_b, (128, C))
        c32[:, 514] = LN_EPS

        in_maps.append({
            "xf": x16_full,
            "x16": np.ascontiguousarray(x16_full[sl_rows]),
            "xr": np.ascontiguousarray(x[sl_rows] + b2[None, :]),
            "em16": np.ascontiguousarray(em_full[sl_rows]),
            "evT16": evT16,
            "decay": np.ascontiguousarray(decay[sl_g]),
            "gidx": gidx,
            "lrow16": lrow16,
            "vals32": vals32,
            "c16": c16,
            "c32": c32,
        })
    plan = dict(CLo_b=CLo_b, CHi_b=CHi_b, CT_b=CT_b, CTmax=CTmax, CTsum=CTsum,
                blk_off=blk_off, call_plan=call_plan, call_off=call_off,
                TOTCALL=TOTCALL)
    return in_maps, plan


# ---------------------------------------------------------------- program
def _build(plan):
    CT_b, CTmax, CTsum = plan["CT_b"], plan["CTmax"], plan["CTsum"]
    CLo_b = plan["CLo_b"]
    blk_off, call_plan, call_off = plan["blk_off"], plan["call_plan"], plan["call_off"]
    TOTCALL = plan["TOTCALL"]

    nc = Bass(num_swdge_queues=4)
    xf_h = nc.dram_tensor("xf", [N, C], dt.float16, kind="ExternalInput")
    x16_h = nc.dram_tensor("x16", [RPC, C], dt.float16, kind="ExternalInput")
    xr_h = nc.dram_tensor("xr", [RPC, C], dt.float32, kind="ExternalInput")
    em_h = nc.dram_tensor("em16", [RPC, K], dt.float16, kind="ExternalInput")
    evT_h = nc.dram_tensor("evT16", [GPC * K, NPG], dt.float16, kind="ExternalInput")
    dec_h = nc.dram_tensor("decay", [GPC, K, C], dt.float32, kind="ExternalInput")
    gidx_h = nc.dram_tensor("gidx", [128, TOTCALL, 64], dt.int16,
                            kind="ExternalInput")
    lrow_h = nc.dram_tensor("lrow16", [128, CTsum], dt.float16,
                            kind="ExternalInput")
    vals_h = nc.dram_tensor("vals32", [128, CTsum], dt.float32,
                            kind="ExternalInput")
    c16_h = nc.dram_tensor("c16", [128, 1792 + CTmax * 128], dt.float16,
                           kind="ExternalInput")
    c32_h = nc.dram_tensor("c32", [128, 516], dt.float32, kind="ExternalInput")
    out_h = nc.dram_tensor("out", [RPC, C], dt.float32, kind="ExternalOutput")

    TS = mybir.AluOpType
    AF = mybir.ActivationFunctionType

    with TileContext(nc) as tc:
        nc.gpsimd.load_library(library_config.mlp)
        nregs = {}
        for nch in sorted({nc_ for p in call_plan for _, nc_ in p}):
            nregs[nch] = nc.gpsimd.to_reg(nch * 128)
        with tc.tile_pool(name="consts", bufs=1) as cp, \
             tc.tile_pool(name="spec", bufs=2) as sp, \
             tc.tile_pool(name="gathp", bufs=12) as gp, \
             tc.tile_pool(name="segp", bufs=2) as sg, \
             tc.tile_pool(name="mlp", bufs=2) as mp, \
             tc.tile_pool(name="ln", bufs=3) as lp, \
             tc.tile_pool(name="ps", bufs=1, space="PSUM") as pp:
            c16 = cp.tile([128, 1792 + CTmax * 128], dt.float16)
            c32 = cp.tile([128, 516], dt.float32)
            nc.sync.dma_start(c16[:], c16_h[:, :])
            nc.sync.dma_start(c32[:], c32_h[:, :])
            W1aT = [[c16[:, (k * 2 + m) * 128:(k * 2 + m + 1) * 128]
                     for m in range(2)] for k in range(2)]
            WfT = [[c16[:, 512 + (k * 2 + m) * 128:512 + (k * 2 + m + 1) * 128]
                    for m in range(2)] for k in range(2)]
            W2T = [c16[:, 1024 + k * 256:1024 + (k + 1) * 256] for k in range(2)]
            ident = c16[:, 1536:1664]
            iota_rep = c16[:, 1792:]
            b1f = [c32[:, m:m + 1] for m in range(2)]
            grep = c32[:, 2:258]
            brep = c32[:, 258:514]
            eps_ap = c32[:, 514:515]
            zero_ap = c32[:, 515:516]

            qn = [0]

            def gather_block(B):
                CT = int(CT_b[B])
                c0, c1 = int(call_off[B]), int(call_off[B + 1])
                gi_t = gp.tile([128, c1 - c0, 64], dt.int16, tag="gidx",
                               name="gi_t")
                nc.sync.dma_start(gi_t[:], gidx_h[:, c0:c1, :])
                lrow_t = gp.tile([128, CTmax, 1], dt.float16, tag="lrow",
                                 name="lrow_t")
                nc.sync.dma_start(
                    lrow_t[:, :CT, 0],
                    lrow_h[:, int(blk_off[B]):int(blk_off[B]) + CT])
                vals_t = gp.tile([128, CTmax], dt.float32, tag="vals",
                                 name="vals_t")
                nc.sync.dma_start(
                    vals_t[:, :CT],
                    vals_h[:, int(blk_off[B]):int(blk_off[B]) + CT])
                tiles = []   # (tile, nch)
                for ci_call, (half, nch) in enumerate(call_plan[B]):
                    gt = gp.tile([128, 8, C], dt.float16, tag="gath",
                                 name="gath_t")
                    src = xf_h[0:HALF, :] if half == 0 else xf_h[HALF:N, :]
                    nc.gpsimd.dma_gather(gt[:, :nch, :], src,
                                         gi_t[:, ci_call, :], nch * 128,
                                         nregs[nch], C, queue_num=qn[0] % 4)
                    qn[0] += 1
                    tiles.append((gt, nch))
                return lrow_t, vals_t, tiles

            def seg_block(B, lrow_t, vals_t, tiles, segT_t):
                CT = int(CT_b[B])
                oh = sg.tile([128, CTmax, 128], dt.float16, tag="oh", name="oh_t")
                nc.vector.tensor_tensor(
                    oh[:, :CT, :],
                    lrow_t[:, :CT, :].broadcast_to([128, CT, 128]),
                    iota_rep[:, :CT * 128].rearrange("p (c j) -> p c j", j=128),
                    TS.is_equal)
                psg = pp.tile([128, C], dt.float32, tag="grad", bufs=2,
                              name="psg")
                ci = 0
                for gt, nch in tiles:
                    for slot in range(nch):
                        nc.scalar.activation(oh[:, ci, :], oh[:, ci, :],
                                             AF.Copy,
                                             scale=vals_t[:, ci:ci + 1])
                        nc.tensor.matmul(psg[:], oh[:, ci, :], gt[:, slot, :],
                                         start=(ci == 0), stop=(ci == CT - 1))
                        ci += 1
                assert ci == CT
                segNM = sg.tile([128, C], dt.float16, tag="segNM", name="segNM")
                nc.scalar.copy(segNM[:], psg[:])
                for h2 in range(2):
                    tp = pp.tile([128, 128], dt.float16, tag="tp", bufs=1,
                                 name="tp")
                    nc.tensor.transpose(tp[:], segNM[:, h2 * 128:(h2 + 1) * 128],
                                        ident)
                    nc.vector.tensor_copy(
                        segT_t[:, h2, (B % 4) * 128:(B % 4 + 1) * 128], tp[:])

            def mlp_group(B, segT_t, diffT):
                n0 = (B - 3) * 128
                goff = n0 % NPG
                gslice = slice(goff, goff + 512)
                hT = mp.tile([128, 2, 512], dt.float16, tag="hT", name="hT")
                for m in range(2):
                    ph = pp.tile([128, 512], dt.float32, tag="h", bufs=2,
                                 name="ph")
                    nc.tensor.matmul(ph[:], W1aT[0][m], diffT[0][:, gslice],
                                     start=True, stop=False)
                    nc.tensor.matmul(ph[:], W1aT[1][m], diffT[1][:, gslice],
                                     start=False, stop=False)
                    for k in range(2):
                        nc.tensor.matmul(ph[:], WfT[k][m], segT_t[:, k, :],
                                         start=False, stop=(k == 1))
                    nc.scalar.activation(hT[:, m, :], ph[:], AF.Relu,
                                         bias=b1f[m], scale=1.0)
                for tt in range(4):
                    nt = B - 3 + tt
                    py = pp.tile([128, C], dt.float32, tag="y", bufs=1,
                                 name="py")
                    for k in range(2):
                        nc.tensor.matmul(py[:], hT[:, k, tt * 128:(tt + 1) * 128],
                                         W2T[k], start=(k == 0), stop=(k == 1))
                    xrt = lp.tile([128, C], dt.float32, tag="xr", name="xrt")
                    nc.sync.dma_start(xrt[:], xr_h[nt * 128:(nt + 1) * 128, :])
                    y = lp.tile([128, C], dt.float32, tag="y", name="y")
                    nc.vector.tensor_add(y[:], py[:], xrt[:])
                    nsum = lp.tile([128, 1], dt.float32, tag="s0", name="s0")
                    nc.vector.tensor_reduce(nsum[:], y[:], mybir.AxisListType.X,
                                            TS.add, negate=True)
                    nmu = lp.tile([128, 1], dt.float32, tag="s1", name="s1")
                    nc.vector.tensor_scalar_mul(nmu[:], nsum[:], 1.0 / C)
                    sq = lp.tile([128, C], dt.float32, tag="sq", name="sq")
                    sqs = lp.tile([128, 1], dt.float32, tag="s2", name="s2")
                    nc.scalar.activation(sq[:], y[:], AF.Square,
                                         bias=zero_ap, accum_out=sqs[:])
                    ex2 = lp.tile([128, 1], dt.float32, tag="s3", name="s3")
                    nc.vector.tensor_scalar_mul(ex2[:], sqs[:], 1.0 / C)
                    mu2 = lp.tile([128, 1], dt.float32, tag="s4", name="s4")
                    nc.vector.tensor_mul(mu2[:], nmu[:], nmu[:])
                    var = lp.tile([128, 1], dt.float32, tag="s5", name="s5")
                    nc.vector.tensor_sub(var[:], ex2[:], mu2[:])
                    sd = lp.tile([128, 1], dt.float32, tag="s6", name="s6")
                    nc.scalar.activation(sd[:], var[:], AF.Sqrt, bias=eps_ap)
                    rstd = lp.tile([128, 1], dt.float32, tag="s7", name="s7")
                    nc.vector.reciprocal(rstd[:], sd[:])
                    yn = lp.tile([128, C], dt.float32, tag="yn", name="yn")
                    nc.vector.tensor_scalar(yn[:], y[:], nmu[:], rstd[:],
                                            TS.add, TS.mult)
                    yg = lp.tile([128, C], dt.float32, tag="yg", name="yg")
                    nc.vector.tensor_mul(yg[:], yn[:], grep)
                    ot = lp.tile([128, C], dt.float32, tag="ot", name="ot")
                    nc.vector.tensor_add(ot[:], yg[:], brep)
                    nc.sync.dma_start(out_h[nt * 128:(nt + 1) * 128, :], ot[:])

            for g in range(GPC):
                em_t = sp.tile([128, 16, K], dt.float16, tag="em", name="em_t")
                nc.sync.dma_start(
                    em_t[:], em_h[g * NPG:(g + 1) * NPG, :].rearrange(
                        "(j p) k -> p j k", p=128))
                xg_t = sp.tile([128, 16, C], dt.float16, tag="xg", name="xg_t")
                nc.sync.dma_start(
                    xg_t[:], x16_h[g * NPG:(g + 1) * NPG, :].rearrange(
                        "(j p) k -> p j k", p=128))
                evT_t = sp.tile([128, NPG], dt.float16, tag="evT", name="evT_t")
                nc.sync.dma_start(evT_t[:], evT_h[g * K:(g + 1) * K, :])
                dec_t = sp.tile([128, C], dt.float32, tag="dec", name="dec_t")
                nc.sync.dma_start(dec_t[:], dec_h[g])

                pxs = pp.tile([128, C], dt.float32, tag="xspec", bufs=1,
                              name="pxs")
                for j in range(16):
                    nc.tensor.matmul(pxs[:], em_t[:, j, :], xg_t[:, j, :],
                                     start=(j == 0), stop=(j == 15))
                xsd = sp.tile([128, C], dt.float16, tag="xsd", name="xsd")
                nc.vector.tensor_mul(xsd[:], pxs[:], dec_t[:])

                diffT = [sp.tile([128, NPG], dt.float16, tag=f"diffT{h2}",
                                 name=f"diffT{h2}")
                         for h2 in range(2)]
                for h2 in range(2):
                    for j in range(4):
                        pd = pp.tile([128, 512], dt.float32, tag="diff", bufs=1,
                                     name="pd")
                        nc.tensor.matmul(pd[:],
                                         xsd[:, h2 * 128:(h2 + 1) * 128],
                                         evT_t[:, j * 512:(j + 1) * 512],
                                         start=True, stop=True)
                        nc.scalar.copy(diffT[h2][:, j * 512:(j + 1) * 512], pd[:])

                for b in range(BPG):
                    B = g * BPG + b
                    if b % 4 == 0:
                        segT_t = sg.tile([128, 2, 512], dt.float16, tag="segT",
                                         name="segT_t")
                    lrow_t, vals_t, tiles = gather_block(B)
                    seg_block(B, lrow_t, vals_t, tiles, segT_t)
                    if b % 4 == 3:
                        mlp_group(B, segT_t, diffT)
    _legalize_waits(nc)
    return nc


# ---------------------------------------------------------------- numpy emu
def emulate_numpy(inputs):
    """Numpy emulation of the device dataflow (fp16 where the device uses
    fp16) — validates host prep + layout logic without hardware."""
    in_maps, plan = _prepare(inputs)
    CT_b, blk_off = plan["CT_b"], plan["blk_off"]
    call_plan, call_off = plan["call_plan"], plan["call_off"]
    outs = []
    for i in range(NCORES):
        m = in_maps[i]
        xf = m["xf"].astype(np.float32)
        em = m["em16"].astype(np.float32).reshape(GPC, NPG, K)
        xg = m["x16"].astype(np.float32).reshape(GPC, NPG, C)
        evT = m["evT16"].astype(np.float32).reshape(GPC, K, NPG)
        xspec = np.einsum('gnk,gnc->gkc', em, xg)
        xsd = (xspec * m["decay"]).astype(np.float16).astype(np.float32)
        diff = np.einsum('gkn,gkc->gnc', evT, xsd).reshape(RPC, C)
        seg = np.zeros((RPC, C), np.float32)
        gidx = m["gidx"]
        lrow = m["lrow16"].astype(np.float32)
        v32 = m["vals32"]
        for b in range(NBLK):
            CT = int(CT_b[b])
            ci = 0
            for ci_call, (half, nch) in enumerate(call_plan[b]):
                call = int(call_off[b]) + ci_call
                for slot in range(nch):
                    jj = np.arange(128) + slot * 128
                    idxs = gidx[jj % 16, call, jj // 16].astype(np.int64)
                    g_rows = xf[idxs + half * HALF]
                    lr = lrow[:, int(blk_off[b]) + ci]
                    vv = v32[:, int(blk_off[b]) + ci]
                    ohv = (lr[:, None] == np.arange(128)[None, :]) * vv[:, None]
                    ohv = ohv.astype(np.float16).astype(np.float32)
                    seg[b * 128:(b + 1) * 128] += ohv.T @ g_rows.astype(
                        np.float16).astype(np.float32)
                    ci += 1
            assert ci == CT
        segT = seg.astype(np.float16).astype(np.float32)
        diffT = diff.astype(np.float16).astype(np.float32)
        c16 = m["c16"].astype(np.float32)
        W1aT = np.zeros((C, C), np.float32)
        WfT = np.zeros((C, C), np.float32)
        for k in range(2):
            for mm_ in range(2):
                W1aT[k * 128:(k + 1) * 128, mm_ * 128:(mm_ + 1) * 128] = \
                    c16[:, (k * 2 + mm_) * 128:(k * 2 + mm_ + 1) * 128]
                WfT[k * 128:(k + 1) * 128, mm_ * 128:(mm_ + 1) * 128] = \
                    c16[:, 512 + (k * 2 + mm_) * 128:512 + (k * 2 + mm_ + 1) * 128]
        W2T = np.concatenate([c16[:, 1024:1280], c16[:, 1280:1536]], 0)
        b1f = np.concatenate([m["c32"][:, 0], m["c32"][:, 1]])
        h = np.maximum(diffT @ W1aT + segT @ WfT + b1f, 0.0)
        h = h.astype(np.float16).astype(np.float32)
        y = m["xr"] + h @ W2T
        mu = y.mean(-1, keepdims=True)
        var = (y * y).mean(-1, keepdims=True) - mu * mu
        g_ = m["c32"][0, 2:258]
        b_ = m["c32"][0, 258:514]
        outs.append((y - mu) / np.sqrt(var + LN_EPS) * g_ + b_)
    return np.concatenate(outs, 0)


# ---------------------------------------------------------------- entry
def kernel(**inputs):
    in_maps, plan = _prepare(inputs)
    nc = _build(plan)
    res = bass_utils.run_bass_kernel_spmd(nc, in_maps,
                                          core_ids=list(range(NCORES)))
    return np.concatenate([res.results[i]["out"] for i in range(NCORES)], 0)



# revision 13
# speedup vs baseline: 1.9767x; 1.9767x over previous
"""DiffusionBlock TRN2 kernel: spectral diffusion + sparse COO gradient op +
MLP + residual LayerNorm, sharded over 8 NeuronCores by node rows.

v2: the sparse segment-sum runs as fp8 one-hot matmuls whose scaled one-hot
operand (vals baked in) is prepared on the host and streamed in as a dense
fp8 grid; gathers fetch fp8 x rows in two big calls per 2-block group.

Self-contained: hardcodes all shapes; builds + compiles a Bass program at
call time (specialized to the edge distribution), runs SPMD on cores 0-7.
"""
import sys
sys.path.insert(0, '/opt/trn_rl_repo')
import numpy as np
import ml_dtypes
import concourse.mybir as mybir
from concourse.bass import Bass
from concourse.tile import TileContext
from concourse import bass_utils, library_config

dt = mybir.dt
FP8 = ml_dtypes.float8_e4m3

# problem dims (hardcoded per contract)
N, C, K, G, E = 65536, 256, 128, 32, 2097152
LN_EPS = 1e-5
NCORES = 8
RPC = N // NCORES          # rows per core = 8192
GPC = G // NCORES          # graphs per core = 4
NPG = N // G               # nodes per graph = 2048
NBLK = RPC // 128          # 128-row blocks per core = 64
BPG = NPG // 128           # blocks per graph = 16
HALF = 32768               # int16 gather index limit
G2 = 4                     # dest blocks per gather group
NGRP = NBLK // G2          # gather groups per core = 16
CPC = 8                    # chunks per gather call (ucode ring cap)


# ---------------------------------------------------------------- BIR fixups
_wspill = [0]


def _legalize_waits(nc):
    """This walrus accepts at most 1 sync-wait per instruction (2 for
    EventSemaphore). Spill extras into EventSemaphore insts inserted just
    before, same engine. Also run codegen_inst_isa_subclasses (Bacc does it,
    raw Bass doesn't) so extended-ISA insts get their raw words."""
    mybir.codegen_inst_isa_subclasses(nc)
    f = nc.m.functions[0]
    for bb in f.blocks:
        out = []
        changed = False
        for ins in bb.instructions:
            si = ins.sync_info
            cap = 2 if ins.opcode == 'EventSemaphore' else 1
            if si is not None and si.on_wait is not None and len(si.on_wait) > cap:
                waits = list(si.on_wait)
                keep, spill = waits[:cap], waits[cap:]
                while spill:
                    batch, spill = spill[:2], spill[2:]
                    _wspill[0] += 1
                    es = mybir.InstEventSemaphore(
                        name=f"WSPILL-{_wspill[0]}", ins=[], outs=[])
                    es.engine = ins.engine
                    es.sync_info = mybir.SyncInfo(on_wait=batch, on_update=[])
                    out.append(es)
                si.on_wait = keep
                changed = True
            out.append(ins)
        if changed:
            bb.instructions = out
    return nc


# ---------------------------------------------------------------- host prep
def _prepare(inputs):
    x = np.asarray(inputs["x"], np.float32)
    evals = np.asarray(inputs["evals_batch"], np.float32)
    evecs = np.asarray(inputs["evecs"], np.float32)
    mass = np.asarray(inputs["mass"], np.float32)
    row = np.asarray(inputs["row"]).astype(np.int64)
    col = np.asarray(inputs["col"]).astype(np.int64)
    vals = np.asarray(inputs["vals"], np.float32)
    t_params = np.asarray(inputs["t_params"], np.float32)
    grad_W = np.asarray(inputs["grad_W"], np.float32)
    grad_b = np.asarray(inputs["grad_b"], np.float32)
    W1 = np.asarray(inputs["W1"], np.float32)
    b1 = np.asarray(inputs["b1"], np.float32)
    W2 = np.asarray(inputs["W2"], np.float32)
    b2 = np.asarray(inputs["b2"], np.float32)
    ln_g = np.asarray(inputs["ln_g"], np.float32)
    ln_b = np.asarray(inputs["ln_b"], np.float32)

    x8_full = x.astype(FP8)                                # gather source
    x16_full = x.astype(np.float16)

    # fold grad_W / grad_b into the second half of W1 (host, fp64 for accuracy)
    W1a = W1[:, :C]
    W1b = W1[:, C:]
    Wfold = (W1b.astype(np.float64) @ grad_W.astype(np.float64)).astype(np.float32)
    b1f_np = b1 + (W1b.astype(np.float64) @ grad_b.astype(np.float64)).astype(np.float32)

    # decay[g,k,c] = exp(-|t_c| * max(ev_gk, 0))
    t = np.abs(t_params)
    ev = np.maximum(evals.reshape(G, K), 0.0)
    decay = np.exp(-ev[:, :, None] * t[None, None, :]).astype(np.float32)  # [G,K,C]

    em_full = (evecs * mass[:, None]).astype(np.float16)   # [N,K]
    ev16_full = evecs.astype(np.float16)

    # ---- edge partitioning by destination row ----
    core_of = row >> 13               # row // 8192
    percore = []
    nlo = np.zeros((NCORES, NBLK), np.int64)
    nhi = np.zeros((NCORES, NBLK), np.int64)
    for i in range(NCORES):
        sel = np.where(core_of == i)[0]
        r = row[sel] - i * RPC
        c_ = col[sel]
        v = vals[sel]
        blk = r >> 7
        lo = c_ < HALF
        lists = []
        for b in range(NBLK):
            m = blk == b
            ilo = np.where(m & lo)[0]
            ihi = np.where(m & ~lo)[0]
            lists.append((ilo, ihi))
            nlo[i, b] = len(ilo)
            nhi[i, b] = len(ihi)
        percore.append((r, c_, v, lists))

    # per-block chunk counts, uniform across cores (SPMD)
    CLo_b = ((nlo.max(0) + 127) // 128).astype(np.int64)
    CHi_b = ((nhi.max(0) + 127) // 128).astype(np.int64)

    # group structure: per group q (blocks G2*q .. G2*q+G2-1):
    #   chunk order [b0_lo .. b3_lo | b0_hi .. b3_hi]
    nlo_q = np.array([sum(CLo_b[G2 * q + rb] for rb in range(G2))
                      for q in range(NGRP)])
    nhi_q = np.array([sum(CHi_b[G2 * q + rb] for rb in range(G2))
                      for q in range(NGRP)])
    CT_q = nlo_q + nhi_q
    CTG_MAX = int(CT_q.max())
    grid_off = np.concatenate([[0], np.cumsum(CT_q)]).astype(np.int64)
    CTsum = int(grid_off[-1])
    # gidx column offsets (16 idx per column, per call)
    idx_cols_q = CT_q * 8           # (nlo+nhi chunks)*128/16
    idx_off = np.concatenate([[0], np.cumsum(idx_cols_q)]).astype(np.int64)
    IDXCOLS = int(idx_off[-1])

    in_maps = []
    for i in range(NCORES):
        r, c_, v, lists = percore[i]
        gidx = np.zeros((16, IDXCOLS), np.int16)
        ohv = np.zeros((128, CTsum, 128), np.float32)
        for q in range(NGRP):
            base = int(grid_off[q])
            icol = int(idx_off[q])
            for half in (0, 1):
                # concatenated padded index stream for this (group, half)
                pos = 0 if half == 0 else int(nlo_q[q])
                stream = []
                for rblk in range(G2):
                    b = G2 * q + rblk
                    idxs_half = lists[b][half]
                    nch = int((CLo_b if half == 0 else CHi_b)[b])
                    cc = c_[idxs_half] - half * HALF
                    rr = r[idxs_half] & 127
                    vv = v[idxs_half]
                    ne = len(cc)
                    # scaled one-hot: ohv[slot, base+pos+ch, lrow] = val
                    ohv[np.arange(ne) % 128, base + pos + np.arange(ne) // 128,
                        rr] = vv
                    ccp = np.zeros(nch * 128, np.int64)
                    ccp[:ne] = cc
                    stream.append(ccp)
                    pos += nch
                stream = np.concatenate(stream)
                # split into calls of <=CPC chunks; idx numbering restarts
                # per call: position jj -> [jj%16, jj//16]
                nq_half = int((nlo_q if half == 0 else nhi_q)[q])
                col0 = icol + (0 if half == 0 else int(nlo_q[q]) * 8)
                off_ = 0
                while off_ < nq_half * 128:
                    n_this = min(CPC * 128, nq_half * 128 - off_)
                    jj = np.arange(n_this)
                    gidx[jj % 16, col0 + off_ // 16 + jj // 16] = \
                        stream[off_:off_ + n_this].astype(np.int16)
                    off_ += n_this
        gidx = np.tile(gidx, (8, 1))   # replicate for the 8 q7 cores
        ohv8 = ohv.reshape(128, CTsum * 128).astype(FP8)

        sl_rows = slice(i * RPC, (i + 1) * RPC)
        sl_g = slice(i * GPC, (i + 1) * GPC)
        evT16 = np.ascontiguousarray(
            ev16_full[sl_rows].reshape(GPC, NPG, K).transpose(0, 2, 1)
        ).reshape(GPC * K, NPG)

        c16 = np.zeros((128, 1664), np.float16)
        off = 0
        W1aT = W1a.T.astype(np.float16)
        WfT = Wfold.T.astype(np.float16)
        for Wt in (W1aT, WfT):
            for k in range(2):
                for m in range(2):
                    c16[:, off:off + 128] = Wt[k * 128:(k + 1) * 128,
                                               m * 128:(m + 1) * 128]
                    off += 128
        W2T = W2.T.astype(np.float16)
        for k in range(2):
            c16[:, off:off + 256] = W2T[k * 128:(k + 1) * 128, :]
            off += 256
        c16[:, off:off + 128] = np.eye(128, dtype=np.float16)
        off += 128
        assert off == 1664

        c32 = np.zeros((128, 516), np.float32)
        c32[:, 0] = b1f_np[:128]
        c32[:, 1] = b1f_np[128:]
        c32[:, 2:258] = np.broadcast_to(ln_g, (128, C))
        c32[:, 258:514] = np.broadcast_to(ln_b, (128, C))
        c32[:, 514] = LN_EPS

        in_maps.append({
            "xf8": x8_full,
            "x16": np.ascontiguousarray(x16_full[sl_rows]),
            "xr": np.ascontiguousarray(x[sl_rows] + b2[None, :]),
            "em16": np.ascontiguousarray(em_full[sl_rows]),
            "evT16": evT16,
            "decay": np.ascontiguousarray(decay[sl_g]),
            "gidx": gidx,
            "ohv": ohv8,
            "c16": c16,
            "c32": c32,
        })
    plan = dict(CLo_b=CLo_b, CHi_b=CHi_b, nlo_q=nlo_q, nhi_q=nhi_q,
                CT_q=CT_q, CTG_MAX=CTG_MAX,
                grid_off=grid_off, idx_off=idx_off, CTsum=CTsum,
                IDXCOLS=IDXCOLS)
    return in_maps, plan


# ---------------------------------------------------------------- program
def _build(plan):
    CLo_b, CHi_b = plan["CLo_b"], plan["CHi_b"]
    nlo_q, nhi_q = plan["nlo_q"], plan["nhi_q"]
    CT_q, CTG_MAX = plan["CT_q"], plan["CTG_MAX"]
    grid_off, idx_off = plan["grid_off"], plan["idx_off"]
    CTsum, IDXCOLS = plan["CTsum"], plan["IDXCOLS"]

    nc = Bass(num_swdge_queues=4)
    xf8_h = nc.dram_tensor("xf8", [N, C], dt.float8e4, kind="ExternalInput")
    x16_h = nc.dram_tensor("x16", [RPC, C], dt.float16, kind="ExternalInput")
    xr_h = nc.dram_tensor("xr", [RPC, C], dt.float32, kind="ExternalInput")
    em_h = nc.dram_tensor("em16", [RPC, K], dt.float16, kind="ExternalInput")
    evT_h = nc.dram_tensor("evT16", [GPC * K, NPG], dt.float16, kind="ExternalInput")
    dec_h = nc.dram_tensor("decay", [GPC, K, C], dt.float32, kind="ExternalInput")
    gidx_h = nc.dram_tensor("gidx", [128, IDXCOLS], dt.int16,
                            kind="ExternalInput")
    ohv_h = nc.dram_tensor("ohv", [128, CTsum * 128], dt.float8e4,
                           kind="ExternalInput")
    c16_h = nc.dram_tensor("c16", [128, 1664], dt.float16, kind="ExternalInput")
    c32_h = nc.dram_tensor("c32", [128, 516], dt.float32, kind="ExternalInput")
    out_h = nc.dram_tensor("out", [RPC, C], dt.float16, kind="ExternalOutput")

    TS = mybir.AluOpType
    AF = mybir.ActivationFunctionType

    with TileContext(nc) as tc:
        nc.gpsimd.load_library(library_config.mlp)
        sizes = set()
        for nq_half in list(nlo_q) + list(nhi_q):
            off = 0
            while off < int(nq_half):
                sizes.add(min(CPC, int(nq_half) - off) * 128)
                off += min(CPC, int(nq_half) - off)
        nregs = {s: nc.gpsimd.to_reg(s) for s in sorted(sizes)}
        with tc.tile_pool(name="consts", bufs=1) as cp, \
             tc.tile_pool(name="spec", bufs=2) as sp, \
             tc.tile_pool(name="gathp", bufs=3) as gp, \
             tc.tile_pool(name="segp", bufs=2) as sg, \
             tc.tile_pool(name="mlp", bufs=2) as mp, \
             tc.tile_pool(name="ln", bufs=3) as lp, \
             tc.tile_pool(name="ps", bufs=1, space="PSUM") as pp:
            c16 = cp.tile([128, 1664], dt.float16)
            c32 = cp.tile([128, 516], dt.float32)
            nc.sync.dma_start(c16[:], c16_h[:, :])
            nc.sync.dma_start(c32[:], c32_h[:, :])
            W1aT = [[c16[:, (k * 2 + m) * 128:(k * 2 + m + 1) * 128]
                     for m in range(2)] for k in range(2)]
            WfT = [[c16[:, 512 + (k * 2 + m) * 128:512 + (k * 2 + m + 1) * 128]
                    for m in range(2)] for k in range(2)]
            W2T = [c16[:, 1024 + k * 256:1024 + (k + 1) * 256] for k in range(2)]
            ident = c16[:, 1536:1664]
            b1f = [c32[:, m:m + 1] for m in range(2)]
            grep = c32[:, 2:258]
            brep = c32[:, 258:514]
            eps_ap = c32[:, 514:515]
            zero_ap = c32[:, 515:516]

            qn = [0]

            def gather_group(q):
                NL, NH = int(nlo_q[q]), int(nhi_q[q])
                icol = int(idx_off[q])
                gi_t = gp.tile([128, CTG_MAX * 8], dt.int16, tag="gidx",
                               name="gi_t")
                nc.sync.dma_start(gi_t[:, :CT_q[q] * 8],
                                  gidx_h[:, icol:icol + int(CT_q[q]) * 8])
                ohv_t = gp.tile([128, CTG_MAX * 128], dt.float8e4, tag="ohv",
                                name="ohv_t")
                goff = int(grid_off[q]) * 128
                nc.sync.dma_start(ohv_t[:, :int(CT_q[q]) * 128],
                                  ohv_h[:, goff:goff + int(CT_q[q]) * 128])
                tiles = {}
                for half, nq_half, src in ((0, NL, xf8_h[0:HALF, :]),
                                           (1, NH, xf8_h[HALF:N, :])):
                    tlist = []
                    col0 = 0 if half == 0 else NL * 8
                    off = 0
                    while off < nq_half:
                        nch = min(CPC, nq_half - off)
                        gt = gp.tile([128, CPC, C], dt.float8e4,
                                     tag=f"g{half}", bufs=14, name="gt")
                        nc.gpsimd.dma_gather(
                            gt[:, :nch, :], src,
                            gi_t[:, col0 + off * 8:col0 + (off + nch) * 8],
                            nch * 128, nregs[nch * 128], C,
                            queue_num=qn[0] % 4)
                        qn[0] += 1
                        tlist.append(gt)
                        off += nch
                    tiles[half] = tlist
                return ohv_t, tiles[0], tiles[1]

            def seg_block(B, ohv_t, lo_tiles, hi_tiles, segT_t):
                q, rblk = B // G2, B % G2
                # chunk positions of this block within the group regions
                lo0 = sum(int(CLo_b[G2 * q + rb]) for rb in range(rblk))
                hi0 = sum(int(CHi_b[G2 * q + rb]) for rb in range(rblk))
                NL = int(nlo_q[q])
                CL, CH = int(CLo_b[B]), int(CHi_b[B])
                CT = CL + CH
                psg = pp.tile([128, C], dt.float32, tag="grad", bufs=2,
                              name="psg")
                ci = 0
                for j in range(CL):
                    pos = lo0 + j          # position within lo region
                    nc.tensor.matmul(psg[:],
                                     ohv_t[:, pos * 128:(pos + 1) * 128],
                                     lo_tiles[pos // CPC][:, pos % CPC, :],
                                     start=(ci == 0), stop=(ci == CT - 1))
                    ci += 1
                for j in range(CH):
                    pos = hi0 + j          # position within hi region
                    nc.tensor.matmul(psg[:],
                                     ohv_t[:, (NL + pos) * 128:(NL + pos + 1) * 128],
                                     hi_tiles[pos // CPC][:, pos % CPC, :],
                                     start=(ci == 0), stop=(ci == CT - 1))
                    ci += 1
                assert ci == CT
                segNM = sg.tile([128, C], dt.float16, tag="segNM", name="segNM")
                nc.scalar.copy(segNM[:], psg[:])
                for h2 in range(2):
                    tp = pp.tile([128, 128], dt.float16, tag="tp", bufs=1,
                                 name="tp")
                    nc.tensor.transpose(tp[:], segNM[:, h2 * 128:(h2 + 1) * 128],
                                        ident)
                    nc.vector.tensor_copy(
                        segT_t[:, h2, (B % 4) * 128:(B % 4 + 1) * 128], tp[:])

            def mlp_group(B, segT_t, diffT):
                n0 = (B - 3) * 128
                goff = n0 % NPG
                gslice = slice(goff, goff + 512)
                hT = mp.tile([128, 2, 512], dt.float16, tag="hT", name="hT")
                for m in range(2):
                    ph = pp.tile([128, 512], dt.float32, tag="h", bufs=2,
                                 name="ph")
                    nc.tensor.matmul(ph[:], W1aT[0][m], diffT[0][:, gslice],
                                     start=True, stop=False)
                    nc.tensor.matmul(ph[:], W1aT[1][m], diffT[1][:, gslice],
                                     start=False, stop=False)
                    for k in range(2):
                        nc.tensor.matmul(ph[:], WfT[k][m], segT_t[:, k, :],
                                         start=False, stop=(k == 1))
                    nc.scalar.activation(hT[:, m, :], ph[:], AF.Relu,
                                         bias=b1f[m], scale=1.0)
                for tt in range(4):
                    nt = B - 3 + tt
                    py = pp.tile([128, C], dt.float32, tag="y", bufs=1,
                                 name="py")
                    for k in range(2):
                        nc.tensor.matmul(py[:], hT[:, k, tt * 128:(tt + 1) * 128],
                                         W2T[k], start=(k == 0), stop=(k == 1))
                    xrt = lp.tile([128, C], dt.float32, tag="xr", name="xrt")
                    nc.sync.dma_start(xrt[:], xr_h[nt * 128:(nt + 1) * 128, :])
                    y = lp.tile([128, C], dt.float32, tag="y", name="y")
                    nc.vector.tensor_add(y[:], py[:], xrt[:])
                    nsum = lp.tile([128, 1], dt.float32, tag="s0", name="s0")
                    nc.vector.tensor_reduce(nsum[:], y[:], mybir.AxisListType.X,
                                            TS.add, negate=True)
                    nmu = lp.tile([128, 1], dt.float32, tag="s1", name="s1")
                    nc.vector.tensor_scalar_mul(nmu[:], nsum[:], 1.0 / C)
                    sq = lp.tile([128, C], dt.float32, tag="sq", name="sq")
                    sqs = lp.tile([128, 1], dt.float32, tag="s2", name="s2")
                    nc.scalar.activation(sq[:], y[:], AF.Square,
                                         bias=zero_ap, accum_out=sqs[:])
                    ex2 = lp.tile([128, 1], dt.float32, tag="s3", name="s3")
                    nc.vector.tensor_scalar_mul(ex2[:], sqs[:], 1.0 / C)
                    mu2 = lp.tile([128, 1], dt.float32, tag="s4", name="s4")
                    nc.vector.tensor_mul(mu2[:], nmu[:], nmu[:])
                    var = lp.tile([128, 1], dt.float32, tag="s5", name="s5")
                    nc.vector.tensor_sub(var[:], ex2[:], mu2[:])
                    sd = lp.tile([128, 1], dt.float32, tag="s6", name="s6")
                    nc.scalar.activation(sd[:], var[:], AF.Sqrt, bias=eps_ap)
                    rstd = lp.tile([128, 1], dt.float32, tag="s7", name="s7")
                    nc.vector.reciprocal(rstd[:], sd[:])
                    yn = lp.tile([128, C], dt.float32, tag="yn", name="yn")
                    nc.vector.scalar_tensor_tensor(
                        yn[:], y[:], nmu[:, 0:1],
                        rstd[:, 0:1].broadcast_to([128, C]),
                        TS.add, TS.mult)
                    yg = lp.tile([128, C], dt.float32, tag="yg", name="yg")
                    nc.vector.tensor_mul(yg[:], yn[:], grep)
                    ot = lp.tile([128, C], dt.float16, tag="ot", name="ot")
                    nc.vector.tensor_add(ot[:], yg[:], brep)
                    nc.sync.dma_start(out_h[nt * 128:(nt + 1) * 128, :], ot[:])

            for g in range(GPC):
                em_t = sp.tile([128, 16, K], dt.float16, tag="em", name="em_t")
                nc.sync.dma_start(
                    em_t[:], em_h[g * NPG:(g + 1) * NPG, :].rearrange(
                        "(j p) k -> p j k", p=128))
                xg_t = sp.tile([128, 16, C], dt.float16, tag="xg", name="xg_t")
                nc.sync.dma_start(
                    xg_t[:], x16_h[g * NPG:(g + 1) * NPG, :].rearrange(
                        "(j p) k -> p j k", p=128))
                evT_t = sp.tile([128, NPG], dt.float16, tag="evT", name="evT_t")
                nc.sync.dma_start(evT_t[:], evT_h[g * K:(g + 1) * K, :])
                dec_t = sp.tile([128, C], dt.float32, tag="dec", name="dec_t")
                nc.sync.dma_start(dec_t[:], dec_h[g])

                pxs = pp.tile([128, C], dt.float32, tag="xspec", bufs=1,
                              name="pxs")
                for j in range(16):
                    nc.tensor.matmul(pxs[:], em_t[:, j, :], xg_t[:, j, :],
                                     start=(j == 0), stop=(j == 15))
                xsd = sp.tile([128, C], dt.float16, tag="xsd", name="xsd")
                nc.vector.tensor_mul(xsd[:], pxs[:], dec_t[:])

                diffT = [sp.tile([128, NPG], dt.float16, tag=f"diffT{h2}",
                                 name=f"diffT{h2}")
                         for h2 in range(2)]
                for h2 in range(2):
                    for j in range(4):
                        pd = pp.tile([128, 512], dt.float32, tag="diff", bufs=1,
                                     name="pd")
                        nc.tensor.matmul(pd[:],
                                         xsd[:, h2 * 128:(h2 + 1) * 128],
                                         evT_t[:, j * 512:(j + 1) * 512],
                                         start=True, stop=True)
                        nc.scalar.copy(diffT[h2][:, j * 512:(j + 1) * 512], pd[:])

                for b in range(BPG):
                    B = g * BPG + b
                    if b % 4 == 0:
                        segT_t = sg.tile([128, 2, 512], dt.float16, tag="segT",
                                         name="segT_t")
                    if B % G2 == 0:
                        ohv_t, lo_tiles, hi_tiles = gather_group(B // G2)
                    seg_block(B, ohv_t, lo_tiles, hi_tiles, segT_t)
                    if b % 4 == 3:
                        mlp_group(B, segT_t, diffT)
    _legalize_waits(nc)
    return nc


# ---------------------------------------------------------------- numpy emu
def emulate_numpy(inputs):
    """Numpy emulation of the device dataflow (fp16/fp8 where the device
    does) — validates host prep + layout logic without hardware."""
    in_maps, plan = _prepare(inputs)
    CLo_b, CHi_b = plan["CLo_b"], plan["CHi_b"]
    nlo_q = plan["nlo_q"]
    grid_off, idx_off = plan["grid_off"], plan["idx_off"]
    CT_q = plan["CT_q"]
    outs = []
    for i in range(NCORES):
        m = in_maps[i]
        xf8 = m["xf8"].astype(np.float32)
        em = m["em16"].astype(np.float32).reshape(GPC, NPG, K)
        xg = m["x16"].astype(np.float32).reshape(GPC, NPG, C)
        evT = m["evT16"].astype(np.float32).reshape(GPC, K, NPG)
        xspec = np.einsum('gnk,gnc->gkc', em, xg)
        xsd = (xspec * m["decay"]).astype(np.float16).astype(np.float32)
        diff = np.einsum('gkn,gkc->gnc', evT, xsd).reshape(RPC, C)
        seg = np.zeros((RPC, C), np.float32)
        gidx = m["gidx"]
        ohv = m["ohv"].astype(np.float32).reshape(128, plan["CTsum"], 128)
        for q in range(NGRP):
            NL = int(nlo_q[q])
            CT = int(CT_q[q])
            icol = int(idx_off[q])
            base = int(grid_off[q])
            # reconstruct gathered tiles
            g_all = np.zeros((128, CT, C), np.float32)
            for half, p0, nchunks in ((0, 0, NL), (1, NL, CT - NL)):
                for chp in range(nchunks):
                    jj = np.arange(128) + chp * 128
                    cols = icol + (NL * 8 if half else 0) + jj // 16
                    idxs = gidx[jj % 16, cols].astype(np.int64)
                    g_all[:, p0 + chp, :] = xf8[idxs + half * HALF]
            # per block accumulate
            for rblk in range(G2):
                B = q * G2 + rblk
                lo0 = sum(int(CLo_b[G2 * q + rb]) for rb in range(rblk))
                hi0 = NL + sum(int(CHi_b[G2 * q + rb]) for rb in range(rblk))
                acc = np.zeros((128, C), np.float32)
                for j in range(int(CLo_b[B])):
                    pos = lo0 + j
                    acc += ohv[:, base + pos, :].T @ g_all[:, pos, :]
                for j in range(int(CHi_b[B])):
                    pos = hi0 + j
                    acc += ohv[:, base + pos, :].T @ g_all[:, pos, :]
                seg[B * 128:(B + 1) * 128] = acc
        segT = seg.astype(np.float16).astype(np.float32)
        diffT = diff.astype(np.float16).astype(np.float32)
        c16 = m["c16"].astype(np.float32)
        W1aT = np.zeros((C, C), np.float32)
        WfT = np.zeros((C, C), np.float32)
        for k in range(2):
            for mm_ in range(2):
                W1aT[k * 128:(k + 1) * 128, mm_ * 128:(mm_ + 1) * 128] = \
                    c16[:, (k * 2 + mm_) * 128:(k * 2 + mm_ + 1) * 128]
                WfT[k * 128:(k + 1) * 128, mm_ * 128:(mm_ + 1) * 128] = \
                    c16[:, 512 + (k * 2 + mm_) * 128:512 + (k * 2 + mm_ + 1) * 128]
        W2T = np.concatenate([c16[:, 1024:1280], c16[:, 1280:1536]], 0)
        b1f = np.concatenate([m["c32"][:, 0], m["c32"][:, 1]])
        h = np.maximum(diffT @ W1aT + segT @ WfT + b1f, 0.0)
        h = h.astype(np.float16).astype(np.float32)
        y = m["xr"] + h @ W2T
        mu = y.mean(-1, keepdims=True)
        var = (y * y).mean(-1, keepdims=True) - mu * mu
        g_ = m["c32"][0, 2:258]
        b_ = m["c32"][0, 258:514]
        o = (y - mu) / np.sqrt(var + LN_EPS) * g_ + b_
        outs.append(o.astype(np.float16).astype(np.float32))
    return np.concatenate(outs, 0)


# ---------------------------------------------------------------- entry
def kernel(**inputs):
    in_maps, plan = _prepare(inputs)
    nc = _build(plan)
    res = bass_utils.run_bass_kernel_spmd(nc, in_maps,
                                          core_ids=list(range(NCORES)))
    return np.concatenate([res.results[i]["out"] for i in range(NCORES)],
                          0).astype(np.float32)
